# revision 17
# baseline (speedup 1.0000x reference)
"""Sparse (routed) MoE feed-forward on 8 TRN2 NeuronCores, v2.

The wall-clock of a dispatch through the axon tunnel is dominated by
host<->device transfer (~45 MB/s, ~75-110 ms fixed per direction), so the
kernel moves the minimum number of bytes per call:

  host:   exact f32 router (x @ router_w.T, softmax, top-2) and per-expert
          slot tables -- 34 MFLOP, ~5 ms, keeps top-2 selection
          bit-comparable to the f32 reference.
  upload: x as int8 [2048,1024] with per-token scales folded into the
          combine-weight table (256 KB/core) + tiny per-expert tables.
          Expert weights are uploaded fp16 ONCE and kept resident on device
          across calls (content-fingerprint cache).
  device: AllGather x shards -> full int8 x; widen to fp16; dispatch =
          PE matmul against an indicator matrix built from the slot table
          (comb weight * dequant scale folded in); SwiGLU + down-proj on the
          CAP routed slots in fp16; scatter back to dense [2048,1024] via a
          second indicator matmul; ReduceScatter(add) over the 8 cores so
          core r ends with the final rows [256r, 256(r+1)).
  download: the fp16 [2048,1024] result, reassembled/cast on host.

Per-call traffic: ~2.2 MB up + 4 MB down + one dispatch RTT.
"""

import numpy as np

P = 128
NTOK = 2048       # full problem size (host)
NST = 4           # pipeline stages: token quarters overlap up/down on the tunnel
NT = NTOK // NST  # 512 tokens per device stage
D = 1024
F = 2048
F2 = 2 * F
E = 8
NSH = NT // E     # 128 tokens per core shard per stage
TCH = NT // P     # 8 token chunks
DC = D // P       # 8
FC = F // P       # 16
CAP = 256         # per-expert per-stage capacity (seed-0 max quarter-load is 145)
CJ = CAP // P     # 2 slot chunks
NMV = CAP // 2    # 128-wide moving chunks for CAP-sized dims

# packed upload blob layout (bytes, per core)
XB = NSH * D                  # int8 x shard
TB = CAP * 4                  # one f32 table
OFF_GIDX = XB                 # gidx row-major [CAP]
OFF_GPC = XB + TB             # gidx wrapped [128, CJ] C-order
OFF_COMB = XB + 2 * TB        # comb row-major [CAP]
NB = XB + 3 * TB
# packed download: int8 rows + per-row f32 scale in 4 trailing byte-columns
DQ = D + 4

_CACHE = {}


# --------------------------------------------------------------------------
# device kernel
# --------------------------------------------------------------------------

def _build():
    import concourse.bacc as bacc
    import concourse.mybir as mybir
    import concourse.tile as tile
    from contextlib import ExitStack

    f32 = mybir.dt.float32
    f16 = mybir.dt.float16
    i8 = mybir.dt.int8
    i32 = mybir.dt.int32
    AF = mybir.ActivationFunctionType
    ALU = mybir.AluOpType
    AX = mybir.AxisListType

    nc = bacc.Bacc("TRN2", target_bir_lowering=False, debug=False, num_devices=8)
    blob_d = nc.dram_tensor("blob", [1, NB], i8, kind="ExternalInput").ap()
    gw_d = nc.dram_tensor("gw", [D, F2], f16, kind="ExternalInput").ap()
    dw_d = nc.dram_tensor("dw", [F, D], f16, kind="ExternalInput").ap()
    # output: per-row int8 quantized result with the f32 dequant scale
    # embedded in the 4 trailing byte-columns of each row. The full result
    # is AllGathered so the host can fetch it from a single device in one
    # D2H stream (cheaper than an 8-shard fetch through the tunnel).
    outq_d = nc.dram_tensor("outq", [NT, DQ], i8, kind="ExternalOutput").ap()
    qloc = nc.dram_tensor("qloc", [NSH, DQ], i8).ap()
    qfull = nc.dram_tensor("qfull", [NT, DQ], i8, addr_space="Shared").ap()

    xsh_d = blob_d[0, 0:XB].rearrange("(p d) -> p d", p=NSH)
    gidx_d = blob_d[0:1, OFF_GIDX:OFF_GIDX + TB].bitcast(f32)
    gpc_d = blob_d[0:1, OFF_GPC:OFF_GPC + TB].bitcast(f32)
    comb_d = blob_d[0:1, OFF_COMB:OFF_COMB + TB].bitcast(f32)

    # collective endpoints (I/O tensors can't be collective operands)
    ag_in = nc.dram_tensor("ag_in", [NSH, D], i8).ap()
    ag_out = nc.dram_tensor("ag_out", [NT, D], i8, addr_space="Shared").ap()
    scat_d = nc.dram_tensor("scat", [NT, D], f16).ap()
    rs_out = nc.dram_tensor("rs_o", [NSH, D], f16).ap()

    gw_r = gw_d.rearrange("(c p) f -> c p f", p=P)    # 8 x [128, 4096]
    dw_r = dw_d.rearrange("(c p) d -> c p d", p=P)    # 16 x [128, 1024]
    xga_r = ag_out.rearrange("(c p) d -> c p d", p=P)  # 16 x [128, 1024]
    scat_r = scat_d.rearrange("(c p) d -> c p d", p=P)

    with tile.TileContext(nc) as tc, ExitStack() as ctx:
        cpool = ctx.enter_context(tc.tile_pool(name="const", bufs=1))
        small = ctx.enter_context(tc.tile_pool(name="small", bufs=1))
        wd = ctx.enter_context(tc.tile_pool(name="wd", bufs=1))
        xgtp = ctx.enter_context(tc.tile_pool(name="xgt", bufs=1))
        hp = ctx.enter_context(tc.tile_pool(name="h", bufs=1))
        ogp = ctx.enter_context(tc.tile_pool(name="og", bufs=1))

        # ---- constants
        ones_row = cpool.tile([1, P], f32, tag="ones_row")
        nc.gpsimd.memset(ones_row[:], 1.0)
        # iota16[p, j] = p + 128*j  (token id of partition p in chunk j)
        it16_i = cpool.tile([P, TCH], i32, tag="it16i")
        nc.gpsimd.iota(it16_i[:], pattern=[[P, TCH]], base=0, channel_multiplier=1)
        iota16 = cpool.tile([P, TCH], f32, tag="iota16")
        nc.vector.tensor_copy(iota16[:], it16_i[:])

        # ---- down-proj weight preload (SBUF-resident for the whole kernel)
        dw_sb = [wd.tile([P, D], f16, tag=f"dw{f}", name=f"dw{f}", bufs=1)
                 for f in range(FC)]
        for f in range(FC):
            nc.sync.dma_start(dw_sb[f][:], dw_r[f])

        # ---- AllGather the token shards into full x (int8)
        nc.sync.dma_start(ag_in, xsh_d)
        nc.gpsimd.collective_compute(
            "AllGather", mybir.AluOpType.bypass,
            replica_groups=[list(range(E))],
            ins=[ag_in], outs=[ag_out],
        )

        # ---- tables
        gidx_row = small.tile([1, CAP], f32, tag="gidxr")
        nc.sync.dma_start(gidx_row[:], gidx_d)
        comb_row = small.tile([1, CAP], f32, tag="combr")
        nc.sync.dma_start(comb_row[:], comb_d)
        # slot-partition wrap: slot s = c*128 + p sits at [p, c]
        # (host packs this layout directly at OFF_GPC)
        gidx_pc = small.tile([P, CJ], f32, tag="gidxpc")
        nc.sync.dma_start(gidx_pc[:], gpc_d[0, :].rearrange("(p c) -> p c", p=P))

        xgT = [xgtp.tile([P, CAP], f16, tag=f"xgt{d}", name=f"xgT{d}", bufs=1)
               for d in range(DC)]

        # ---- phase B: dispatch xgT[d, s] = sum_t x[t, d] * comb[s]*[gidx[s]==t]
        with tc.tile_pool(name="xs8", bufs=1) as x8p, \
             tc.tile_pool(name="xs", bufs=1) as xsp, \
             tc.tile_pool(name="ind2", bufs=1) as i2p, \
             tc.tile_pool(name="btmp", bufs=2) as btp, \
             tc.tile_pool(name="pbc", bufs=2, space="PSUM") as pbc, \
             tc.tile_pool(name="pb", bufs=4, space="PSUM") as pb:
            # broadcast gidx/comb rows across partitions via PE outer product
            gidx_bc = small.tile([P, CAP], f32, tag="gidxbc")
            comb_bc = small.tile([P, CAP], f32, tag="combbc")
            for h in range(2):
                sl = slice(h * NMV, (h + 1) * NMV)
                pg = pbc.tile([P, NMV], f32, tag="bc")
                nc.tensor.matmul(pg[:], ones_row[:], gidx_row[:, sl],
                                 start=True, stop=True)
                nc.vector.tensor_copy(gidx_bc[:, sl], pg[:])
                pc2 = pbc.tile([P, NMV], f32, tag="bc")
                nc.tensor.matmul(pc2[:], ones_row[:], comb_row[:, sl],
                                 start=True, stop=True)
                nc.vector.tensor_copy(comb_bc[:, sl], pc2[:])

            ind2 = [i2p.tile([P, CAP], f16, tag=f"i2_{t}", name=f"ind2_{t}",
                             bufs=1) for t in range(TCH)]
            for t in range(TCH):
                eq = btp.tile([P, CAP], f32, tag="eq")
                nc.vector.tensor_scalar(
                    eq[:], gidx_bc[:], iota16[:, t:t + 1], None,
                    op0=ALU.is_equal)
                nc.vector.tensor_tensor(ind2[t][:], eq[:], comb_bc[:],
                                        op=ALU.mult)

            # int8 -> fp16 widen (int values <= 127 are exact in fp16)
            xs8 = [x8p.tile([P, D], i8, tag=f"x8_{t}", name=f"x8_{t}", bufs=1)
                   for t in range(TCH)]
            xs = [xsp.tile([P, D], f16, tag=f"xs{t}", name=f"xs{t}", bufs=1)
                  for t in range(TCH)]
            for t in range(TCH):
                nc.sync.dma_start(xs8[t][:], xga_r[t])
                nc.vector.tensor_copy(xs[t][:], xs8[t][:])

            for d in range(DC):
                for mv in range(2):
                    ps = pb.tile([P, NMV], f32, tag="pb")
                    for t in range(TCH):
                        nc.tensor.matmul(
                            ps[:],
                            xs[t][:, d * P:(d + 1) * P],
                            ind2[t][:, mv * NMV:(mv + 1) * NMV],
                            start=(t == 0), stop=(t == TCH - 1),
                        )
                    nc.vector.tensor_copy(xgT[d][:, mv * NMV:(mv + 1) * NMV],
                                          ps[:])

        # ---- phase C: gate_up + SwiGLU -> h[fc] [128f, CAP] fp16
        hh = [hp.tile([P, CAP], f16, tag=f"h{f}", name=f"h{f}", bufs=1)
              for f in range(FC)]
        with tc.tile_pool(name="wg", bufs=1) as wg, \
             tc.tile_pool(name="pgu", bufs=8, space="PSUM") as pgu, \
             tc.tile_pool(name="sgt", bufs=4) as sgp:
            gw_sb = [wg.tile([P, F2], f16, tag=f"gw{d}", name=f"gw{d}", bufs=1)
                     for d in range(DC)]
            for d in range(DC):
                nc.sync.dma_start(gw_sb[d][:], gw_r[d])
            for fc in range(FC):
                psg = [pgu.tile([P, NMV], f32, tag="gu", name=f"psg{fc}_{i}")
                       for i in range(2)]
                psu = [pgu.tile([P, NMV], f32, tag="gu", name=f"psu{fc}_{i}")
                       for i in range(2)]
                for d in range(DC):
                    gsl = gw_sb[d][:, fc * P:(fc + 1) * P]
                    usl = gw_sb[d][:, F + fc * P:F + (fc + 1) * P]
                    for mv in range(2):
                        msl = slice(mv * NMV, (mv + 1) * NMV)
                        nc.tensor.matmul(psg[mv][:], gsl, xgT[d][:, msl],
                                         start=(d == 0), stop=(d == DC - 1))
                        nc.tensor.matmul(psu[mv][:], usl, xgT[d][:, msl],
                                         start=(d == 0), stop=(d == DC - 1))
                for mv in range(2):
                    msl = slice(mv * NMV, (mv + 1) * NMV)
                    nc.vector.tensor_scalar(psg[mv][:], psg[mv][:], -10.0, 10.0,
                                            op0=ALU.max, op1=ALU.min)
                    sg = sgp.tile([P, NMV], f32, tag="sg")
                    nc.scalar.activation(sg[:], psg[mv][:], AF.Sigmoid)
                    nc.vector.tensor_tensor(sg[:], sg[:], psg[mv][:],
                                            op=ALU.mult)
                    nc.vector.tensor_tensor(hh[fc][:, msl], sg[:], psu[mv][:],
                                            op=ALU.mult)

        # ---- phase D: down-proj -> og[c] [128s, 1024] fp16
        og = [ogp.tile([P, D], f16, tag=f"og{c}", name=f"og{c}", bufs=1)
              for c in range(CJ)]
        with tc.tile_pool(name="pdn", bufs=4, space="PSUM") as pdn:
            for c in range(CJ):
                for dh in range(2):
                    po = pdn.tile([P, 512], f32, tag="dn")
                    for fc in range(FC):
                        nc.tensor.matmul(
                            po[:],
                            hh[fc][:, c * P:(c + 1) * P],
                            dw_sb[fc][:, dh * 512:(dh + 1) * 512],
                            start=(fc == 0), stop=(fc == FC - 1),
                        )
                    nc.scalar.copy(og[c][:, dh * 512:(dh + 1) * 512], po[:])

        # ---- phase E: scatter back to dense token rows
        with tc.tile_pool(name="indsc", bufs=1) as iscp, \
             tc.tile_pool(name="scat", bufs=3) as scp, \
             tc.tile_pool(name="psc", bufs=4, space="PSUM") as psc:
            # iota_tok[p, t] = t for t in 0..2047 (same on every partition)
            with tc.tile_pool(name="itok", bufs=1) as itp:
                itok_i = itp.tile([P, NT], i32, tag="itoki")
                nc.gpsimd.iota(itok_i[:], pattern=[[1, NT]], base=0,
                               channel_multiplier=0)
                iota_tok = iscp.tile([P, NT], f32, tag="iotat")
                nc.vector.tensor_copy(iota_tok[:], itok_i[:])
            ind_sc = [iscp.tile([P, NT], f16, tag=f"isc{c}", name=f"isc{c}",
                                bufs=1) for c in range(CJ)]
            for c in range(CJ):
                nc.vector.tensor_scalar(
                    ind_sc[c][:], iota_tok[:], gidx_pc[:, c:c + 1], None,
                    op0=ALU.is_equal)
            for t in range(TCH):
                sc = scp.tile([P, D], f16, tag="sc")
                for dh in range(2):
                    ps = psc.tile([P, 512], f32, tag="ps")
                    for c in range(CJ):
                        nc.tensor.matmul(
                            ps[:],
                            ind_sc[c][:, t * P:(t + 1) * P],
                            og[c][:, dh * 512:(dh + 1) * 512],
                            start=(c == 0), stop=(c == CJ - 1),
                        )
                    nc.vector.tensor_copy(sc[:, dh * 512:(dh + 1) * 512], ps[:])
                nc.sync.dma_start(scat_r[t], sc[:])

        # ---- phase F: ReduceScatter(add) -> this core's final 256 rows
        nc.gpsimd.collective_compute(
            "ReduceScatter", mybir.AluOpType.add,
            replica_groups=[list(range(E))],
            ins=[scat_d], outs=[rs_out],
        )

        # ---- phase G: per-row int8 quantization of the result rows
        PQ = min(P, NSH)
        rs_r = rs_out.rearrange("(c p) d -> c p d", p=PQ)
        outq_r = qloc.rearrange("(c p) d -> c p d", p=PQ)
        with tc.tile_pool(name="qp", bufs=2) as qp:
            for c in range(NSH // PQ):
                rt = qp.tile([PQ, D], f16, tag="rt")
                nc.sync.dma_start(rt[:], rs_r[c])
                ab = qp.tile([PQ, D], f32, tag="ab")
                nc.scalar.activation(ab[:], rt[:], AF.Abs)
                am = qp.tile([PQ, 1], f32, tag="am")
                nc.vector.reduce_max(am[:], ab[:], axis=AX.X)
                nc.vector.tensor_scalar(am[:], am[:], 1e-30, None, op0=ALU.max)
                si = qp.tile([PQ, 1], f32, tag="si")
                nc.vector.reciprocal(si[:], am[:])
                nc.vector.tensor_scalar(si[:], si[:], 126.5, None, op0=ALU.mult)
                qt = qp.tile([PQ, DQ], i8, tag="qt")
                nc.vector.tensor_scalar(qt[:, 0:D], rt[:], si[:, 0:1], None,
                                        op0=ALU.mult)
                sct = qp.tile([PQ, 1], f32, tag="sct")
                nc.vector.tensor_scalar(sct[:], am[:], 1.0 / 126.5, None,
                                        op0=ALU.mult)
                nc.vector.tensor_copy(qt[:, D:DQ], sct[:].bitcast(i8))
                nc.sync.dma_start(outq_r[c], qt[:])

        # ---- phase H: AllGather the quantized rows; host reads one device
        nc.gpsimd.collective_compute(
            "AllGather", mybir.AluOpType.bypass,
            replica_groups=[list(range(E))],
            ins=[qloc], outs=[qfull],
        )
        nc.sync.dma_start(outq_d, qfull)
    return nc


# --------------------------------------------------------------------------
# dispatcher: shard_map over 8 cores with device-resident weight cache
# --------------------------------------------------------------------------

class _Dispatcher:
    def __init__(self, nc):
        import jax
        from jax.sharding import Mesh, PartitionSpec, NamedSharding
        import concourse.mybir as mybir
        from concourse.bass2jax import (
            _bass_exec_p, install_neuronx_cc_hook, partition_id_tensor)

        install_neuronx_cc_hook()
        assert nc.dbg_addr is None or not nc.dbg_callbacks
        partition_name = (nc.partition_id_tensor.name
                          if nc.partition_id_tensor else None)

        in_names, out_names, out_avals, zero_shapes = [], [], [], []
        for alloc in nc.m.functions[0].allocations:
            if not isinstance(alloc, mybir.MemoryLocationSet):
                continue
            name = alloc.memorylocations[0].name
            if alloc.kind == "ExternalInput":
                if name != partition_name:
                    in_names.append(name)
            elif alloc.kind == "ExternalOutput":
                shape = tuple(alloc.tensor_shape)
                dtype = mybir.dt.np(alloc.dtype)
                out_names.append(name)
                out_avals.append(jax.core.ShapedArray(shape, dtype))
                zero_shapes.append((shape, dtype))
        self.in_names = list(in_names)
        self.out_names = list(out_names)
        self.zero_shapes = zero_shapes
        n_params = len(in_names)
        n_outs = len(out_names)
        all_names = in_names + out_names
        if partition_name is not None:
            all_names = all_names + [partition_name]

        devices = jax.devices()[:E]
        self.mesh = Mesh(np.asarray(devices), ("core",))
        self.sharding = NamedSharding(self.mesh, PartitionSpec("core"))

        def _body(*args):
            operands = list(args)
            if partition_name is not None:
                operands.append(partition_id_tensor())
            outs = _bass_exec_p.bind(
                *operands,
                out_avals=tuple(out_avals),
                in_names=tuple(all_names),
                out_names=tuple(out_names),
                lowering_input_output_aliases=(),
                sim_require_finite=True,
                sim_require_nnan=True,
                nc=nc,
            )
            return tuple(outs)

        from jax.experimental.shard_map import shard_map
        in_specs = (PartitionSpec("core"),) * (n_params + n_outs)
        out_specs = (PartitionSpec("core"),) * n_outs
        # No donation: the NEFF writes its outputs into the custom-call
        # result buffers, so the zero "output seed" arrays stay untouched
        # and one persistent device-resident copy can be reused every call.
        self.fn = jax.jit(
            shard_map(_body, mesh=self.mesh, in_specs=in_specs,
                      out_specs=out_specs, check_rep=False),
            keep_unused=True,
        )
        self.compiled = None
        self._jax = jax

    def put(self, tree):
        return self._jax.device_put(tree, self.sharding)

    def run(self, by_name, zeros):
        args = [by_name[n] for n in self.in_names] + list(zeros)
        # AOT-compiled executable skips the jit-call dispatch overhead
        # (~4-8 ms/call), which matters when several pipelined stage
        # dispatches are issued back-to-back.
        if self.compiled is None:
            try:
                self.compiled = self.fn.lower(*args).compile()
            except Exception:
                self.compiled = False
        if self.compiled:
            try:
                return self.compiled(*args)
            except Exception:
                pass
        return self.fn(*args)


def _get_state():
    if "disp" not in _CACHE:
        nc = _build()
        nc.compile()
        _CACHE["disp"] = _Dispatcher(nc)
    return _CACHE["disp"]


# --------------------------------------------------------------------------
# host side: routing + table construction
# --------------------------------------------------------------------------

def _route(x, router_w):
    """Exact f32 routing identical to the reference's math (any row count)."""
    lg = x @ router_w.T                         # [n, E] f32
    m = lg.max(-1, keepdims=True)
    p = np.exp(lg - m)
    p /= p.sum(-1, keepdims=True)
    top2 = np.argpartition(-p, 1, axis=-1)[:, :2]   # [n, 2] (order-agnostic)
    rows = np.arange(x.shape[0])
    comb = np.zeros_like(p)
    comb[rows, top2[:, 0]] = p[rows, top2[:, 0]]
    comb[rows, top2[:, 1]] = p[rows, top2[:, 1]]
    return comb, top2


def _tables(comb, top2, scale, base):
    """Per-expert slot tables for one stage block (comb/top2/scale are
    stage-local [NT] arrays): stage-local token id (pad=NT) and combine
    weight (with the int8 dequant scale of the token folded in).
    Overflow records global token ids plus their combine weights."""
    gidx = np.full((E, CAP), float(NT), np.float32)
    cw = np.zeros((E, CAP), np.float32)
    overflow = []
    for e in range(E):
        toks = np.where((top2 == e).any(axis=1))[0]
        if len(toks) > CAP:
            overflow.append((e, base + toks[CAP:], comb[toks[CAP:], e]))
            toks = toks[:CAP]
        gidx[e, :len(toks)] = toks.astype(np.float32)
        cw[e, :len(toks)] = comb[toks, e] * scale[toks]
    return gidx, cw, overflow


def _overflow_fix(x, gate_up_w, down_w, overflow, out):
    for e, toks, w in overflow:
        xin = x[toks] * w[:, None]
        gu = xin @ gate_up_w[e]
        gate = np.clip(gu[:, :F], -10.0, 10.0)
        h = gate / (1.0 + np.exp(-gate)) * gu[:, F:]
        out[toks] += h @ down_w[e]


def _fingerprint(arr):
    a = arr.reshape(-1)
    step = max(1, a.size // 4096)
    return (arr.shape, arr.dtype.str, a[::step].tobytes())


def kernel(x, router_w, gate_up_w, down_w):
    """Full-input entry point with a rebuild-and-retry guard: a transient
    device error (e.g. NRT_EXEC_UNIT_UNRECOVERABLE through the axon proxy)
    drops all cached device state and retries from scratch."""
    last = None
    for attempt in range(4):
        try:
            return _kernel_once(x, router_w, gate_up_w, down_w)
        except Exception as e:   # noqa: BLE001 - deliberate broad retry
            last = e
            _CACHE.clear()
            import gc
            import time
            gc.collect()
            # a dead axon worker wedges the PJRT client; a fresh backend
            # re-establishes the tunnel. The worker itself may take tens
            # of seconds to come back, hence the escalating backoff.
            try:
                import jax
                jax.clear_caches()
            except Exception:
                pass
            for clear in ("jax.extend.backend.clear_backends",
                          "jax._src.api.clear_backends"):
                try:
                    mod, fn = clear.rsplit(".", 1)
                    import importlib
                    getattr(importlib.import_module(mod), fn)()
                    break
                except Exception:
                    continue
            time.sleep((3.0, 12.0, 30.0, 0.0)[attempt])
    raise last


def _kernel_once(x, router_w, gate_up_w, down_w):
    x = np.ascontiguousarray(x, dtype=np.float32)
    router_w = np.asarray(router_w, dtype=np.float32)
    disp = _get_state()

    # static weights: upload once, keep resident on device
    wkey = (_fingerprint(np.asarray(gate_up_w)), _fingerprint(np.asarray(down_w)))
    if _CACHE.get("wkey") != wkey:
        gw = np.asarray(gate_up_w, np.float32).astype(np.float16)
        dw = np.asarray(down_w, np.float32).astype(np.float16)
        _CACHE["gw_dev"], _CACHE["dw_dev"] = disp.put(
            (gw.reshape(E * D, F2), dw.reshape(E * F, D)))
        _CACHE["wkey"] = wkey
    if "zeros_dev" not in _CACHE:
        _CACHE["zeros_dev"] = [
            disp.put(np.zeros((E * s[0],) + s[1:], d))
            for s, d in disp.zero_shapes]

    tmp = _CACHE.get("qtmp")
    if tmp is None or tmp.shape != (NT, D):
        tmp = _CACHE["qtmp"] = np.empty((NT, D), np.float32)

    # Pipelined stages over token blocks: dispatches are issued async
    # back-to-back, so a stage's result download overlaps the next stage's
    # upload on the full-duplex tunnel. ALL host work (routing, int8 quant,
    # table/blob packing) runs per stage inside the loop, so stage 0's
    # dispatch goes out after ~1/NST of the host pre-work and later stages'
    # host work overlaps in-flight transfers.
    shards, overflow = [], []
    for st in range(NST):
        base = st * NT
        xb = x[base:base + NT]
        comb, top2 = _route(xb, router_w)
        absmax = np.abs(xb).max(axis=1)
        scale = (absmax * (1.0 / 127.0) + 1e-30).astype(np.float32)
        gidx, cw, ovf = _tables(comb, top2, scale, base)
        overflow.extend(ovf)
        # per-token symmetric int8 quantization of this stage's tokens;
        # |x|/scale <= 127 by construction so rint stays in int8 range
        np.multiply(xb, (1.0 / scale)[:, None], out=tmp)
        np.rint(tmp, out=tmp)
        xq = tmp.astype(np.int8)
        blob = np.empty((E, NB), np.int8)
        blob[:, :XB] = xq.reshape(E, XB)
        blob[:, OFF_GIDX:OFF_GIDX + TB] = gidx.view(np.int8)
        gpc = np.ascontiguousarray(
            gidx.reshape(E, CJ, P).transpose(0, 2, 1))  # [E, 128, CJ]
        blob[:, OFF_GPC:OFF_GPC + TB] = gpc.reshape(E, TB // 4).view(np.int8)
        blob[:, OFF_COMB:OFF_COMB + TB] = cw.view(np.int8)
        outs = disp.run(
            {"blob": blob, "gw": _CACHE["gw_dev"], "dw": _CACHE["dw_dev"]},
            _CACHE["zeros_dev"],
        )
        # every device holds the full stage result; rotate the source device
        # per stage so fetches spread across per-device D2H queues
        sh = outs[0].addressable_shards[st].data
        try:
            sh.copy_to_host_async()
        except Exception:
            pass
        shards.append(sh)

    out = np.empty((NTOK, D), np.float32)
    for st, sh in enumerate(shards):
        buf = np.asarray(sh)                            # [NT, 1028] int8
        sc = np.ascontiguousarray(buf[:, D:DQ]).view(np.float32)
        # fused dequant straight into the output block (no f32 temp)
        np.multiply(buf[:, :D], sc, out=out[st * NT:(st + 1) * NT])
    if overflow:
        _overflow_fix(x, np.asarray(gate_up_w, np.float32),
                      np.asarray(down_w, np.float32), overflow, out)
    return out
